# revision 48
# baseline (speedup 1.0000x reference)
"""Trainium2 Bass kernel for Conv2D_DT (distance-transform conv).

d(n,o,h,w) = || patch(n,:,h,w) - W[o,:] ||_2  with 3x3/pad1 im2col patches.

Strategy (8 NeuronCores, data-parallel over batch, 4 images/core):
  - the compute-heavy cross term -2 p.w runs as fp8 DoubleRow matmuls at
    the PE's full fp8 rate: each matmul contracts TWO 3x3 taps at once
    (k-tile pair), using hand-built access patterns whose k-tile dim
    strides between the two shifted x windows.  9 taps -> 4 tap-pair
    matmuls + 1 final matmul that pairs tap8 with the ||p||^2 term: its
    second k-tile reads a precomputed b' = ||p||^2 - 576 row (partitions
    0-2/64-66, one-hot weight rows of 16), so the whole quadratic form
    accumulates in PSUM in 5 DoubleRow matmuls per chunk-image.
  - b' = 3x3-box(channel-sum(x^2)) - 576 is computed on host (f32,
    exact) and shipped as a tiny 3-term fp8 expansion (6 partitions per
    pair); the remaining 122 partitions of the b' plane only need to be
    FINITE (their lhsT rows are zero), so they are memset on the
    otherwise-idle vector/gpsimd engines instead of DMAing zeros.
  - image pairs: image A channels on SBUF partitions 0-63, B on 64-127.
  - input DMAs are split between the two hardware DGE queues (sync and
    scalar; gpsimd's software path only carries tiny pair-1 b' rows) so
    their ~0.65us-per-issue cost is paid in parallel right after the
    NEFF preamble; outputs issue on sync (idle during the stream).
    The pair-0 b' rows lead the scalar queue: job 0's slot-4 matmuls
    wait on them, and if they model late the Tile scheduler defers
    those stop matmuls deep into job 1, which inflates the first
    epilogue act's wait threshold by ~1.5us.
  - the f32 w2 bias is byte-packed onto the tail of the weights tensor
    (one DMA, one completion semaphore) because a [128,1] f32 transfer
    is 128 straggling 4-byte packets that would gate the first act.
  - warm-up matmuls on a zeroed scratch tile keep the PE busy from right
    after the preamble until the first x rows land, so the HAM clock
    gate reaches 8/8 before (or shortly after) the real tap stream.
  - a dummy Sqrt activation at the head pulls the lazy ~1.3us ACT table
    load off the first epilogue's critical path.
  - the work is cut into 7 UNIFORM psum jobs of 20 matmuls / one
    full-size act (psum tile [128,4,512] = 4 banks, ring of 2): the two
    odd leftover chunks of the two image pairs merge into one shared
    job, so no short job ever bubbles the psum ring.  The shared job
    runs LAST: its output is naturally two half-size DMAs, which drain
    in parallel on the sync + scalar hardware queues at the tail.
  - epilogue: ONE ScalarE op per job covering both images:
    out = Sqrt(psum/16 + (w2+576)) -> bf16, then one output DMA per
    contiguous chunk run.
"""

import sys

_REPO = "/opt/trn_rl_repo"
if _REPO not in sys.path:
    sys.path.insert(0, _REPO)

import ml_dtypes
import numpy as np

import concourse.bass as bass  # noqa: F401
import concourse.mybir as mybir
import concourse.tile as tile
from concourse import bacc
from concourse.bass_utils import run_bass_kernel_spmd

# Problem geometry (hardcoded per harness contract).
N, C, H, W_DIM, O = 32, 64, 56, 56, 128
NCORES = 8
NL = N // NCORES  # images per core
NPAIR = NL // 2  # image pairs per core
HP = WP = 58  # zero-padded spatial dims
RCH = 8  # output rows per PSUM chunk slot
WSCALE = 16.0  # fp8 W pre-scale; undone by epilogue scale=1/WSCALE
BCENTER = 576.0  # E[||p||^2]; recentering keeps b' in fp8 range
NWARM = 26  # PE warm-up matmuls (HAM clock-gate ramp)
NSLOT = 5  # DoubleRow k-tile pairs: (t0,t1)(t2,t3)(t4,t5)(t6,t7)(t8,b)

F32 = mybir.dt.float32
BF16 = mybir.dt.bfloat16
U32 = mybir.dt.uint32
FP8 = mybir.dt.float8e4
NP_FP8 = ml_dtypes.float8_e4m3

# PSUM jobs: each is a list of (pair, chunk) entries sharing one 4-bank
# psum tile + one epilogue act.  The two odd leftover chunks (pair-0
# chunk 6 and pair-1 chunk 0) are merged into ONE shared k=2 job, so all
# 7 jobs are uniform 20-matmul / full-size-act units - no short-group
# pipeline bubbles in the psum ring.
JOBS = (
    ((0, 0), (0, 1)),
    ((0, 2), (0, 3)),
    ((0, 4), (0, 5)),
    ((1, 1), (1, 2)),
    ((1, 3), (1, 4)),
    ((1, 5), (1, 6)),
    ((0, 6), (1, 0)),
)
# tap-A (kh,kw) of each DoubleRow pair, and the k-tile-dim element stride
# from tap A's window to tap B's (tap index t=(kh,kw) offset = kh*WP+kw;
# slot 4 jumps from x (slot 0) to the b' plane (slot 1) of the same tile)
TAPA = ((0, 0), (0, 2), (1, 1), (2, 0), (2, 2))
DELTA = (1, WP - 2, 1, 1, HP * WP)

_PROGRAM = None


def _pair_rhs(t, half, lh, s):
    """rhs AP [64, 2, RCH, 56] for DoubleRow pair s: dim1 walks from tap A's
    shifted window to tap B's (stride DELTA[s]) inside tile t (free dims
    flat: slot*HP*WP + row*WP + col)."""
    kh, kw = TAPA[s]
    off = (lh + kh) * WP + kw
    ap = t[half, 0, off : off + 56]
    l = ap.ap
    l.insert(1, (DELTA[s], 2))
    l.insert(2, (WP, RCH))
    return ap


def _build_program():
    nc = bacc.Bacc(
        "TRN2",
        target_bir_lowering=False,
        debug=False,
        enable_asserts=False,
        num_devices=NCORES,
    )
    xq = nc.dram_tensor("xq", [NPAIR, 128, HP * WP], FP8, kind="ExternalInput")
    bq = nc.dram_tensor("bq", [NPAIR, 2, 3, HP * WP], FP8, kind="ExternalInput")
    # weights + per-partition f32 w2 bias packed into one tensor so the
    # bias lands with the (early, sync-queue) weight DMA instead of as
    # 128 straggling 4-byte packets that gate the first epilogue act
    lw = nc.dram_tensor("lw", [128, NSLOT * 256 + 4], FP8, kind="ExternalInput")
    out = nc.dram_tensor("out", [NPAIR, O, 2, H, W_DIM], BF16, kind="ExternalOutput")

    DR = mybir.MatmulPerfMode.DoubleRow
    SQRT = mybir.ActivationFunctionType.Sqrt

    with tile.TileContext(nc) as tc:
        with (
            tc.tile_pool(name="const", bufs=1) as cpool,
            tc.tile_pool(name="imgs", bufs=2) as ipool,
            tc.tile_pool(name="outs", bufs=6) as opool,
            tc.tile_pool(name="psum", bufs=2, space="PSUM") as ppool,
        ):
            # scratch for PE warm-up (zeroed so no NaNs reach the PE)
            scr = cpool.tile([128, 128], FP8)
            nc.vector.memset(scr[:], 0)
            dum = cpool.tile([128, 1], F32)
            dumo = cpool.tile([128, 1], F32)

            xst = []
            for p in range(NPAIR):
                xsp = ipool.tile([128, 2, HP * WP], FP8, tag="xs")
                xst.append(xsp)

            lwt = cpool.tile([128, NSLOT * 256 + 4], FP8)

            def _lhs(half, sl):
                ap = lwt[half, 256 * sl : 256 * (sl + 1)]
                l = ap.ap
                l.pop(1)
                l.insert(1, (128, 2))
                l.insert(2, (1, 128))
                return ap

            def _w2v():
                return lwt[:, NSLOT * 256 : NSLOT * 256 + 4].bitcast(F32)

            # Input marshaling. Only sync and scalar have hardware DGE
            # queues (gpsimd DMAs descriptor-gen in software - slow), so
            # the big transfers ride those two, issue cost in parallel.
            # b'-plane zero fills emitted BEFORE the b' row DMAs: program
            # order is dependency order, the rows must land after the fill.
            nc.vector.memset(xst[0][0:64, 1, :].bitcast(U32), 0)
            nc.vector.memset(xst[0][64:128, 1, :].bitcast(U32), 0)
            nc.vector.memset(dum[:], 0)
            nc.vector.memset(xst[1][0:64, 1, :].bitcast(U32), 0)
            nc.vector.memset(xst[1][64:128, 1, :].bitcast(U32), 0)
            # sync: weights first (gate the first LDWEIGHTS), then pair-1 x
            nc.sync.dma_start(out=lwt[:], in_=lw[:, :])
            nc.sync.dma_start(out=xst[1][:, 0, :], in_=xq[1, :, :])
            # scalar: pair-0 x (first-group rows first), w2 bias; then a
            # dummy Sqrt so the ~1.3us ACT table load runs during the DMA
            # head, not before the first epilogue act
            # pair-0 b' rows lead their queues: job 0's slot-4 stop
            # matmuls wait on them, and if they model late the Tile
            # scheduler defers those stop MMs deep into job 1, stalling
            # the act chain.  Half A leads scalar; half B leads gpsimd
            # (software DGE, but it starts early and models early).
            nc.scalar.dma_start(out=xst[0][0:3, 1, :], in_=bq[0, 0, :, :])
            nc.gpsimd.dma_start(out=xst[0][64:67, 1, :], in_=bq[0, 1, :, :])
            nc.scalar.dma_start(
                out=xst[0][:, 0, 0 : 20 * WP], in_=xq[0, :, 0 : 20 * WP]
            )
            nc.scalar.dma_start(
                out=xst[0][:, 0, 20 * WP : HP * WP], in_=xq[0, :, 20 * WP : HP * WP]
            )
            nc.scalar.activation(
                out=dumo[:], in_=dum[:], func=SQRT, bias=dum[:], scale=1.0 / WSCALE
            )
            # gpsimd: pair-1 b' rows (tiny; software path is fine here)
            nc.gpsimd.dma_start(out=xst[1][0:3, 1, :], in_=bq[1, 0, :, :])
            nc.gpsimd.dma_start(out=xst[1][64:67, 1, :], in_=bq[1, 1, :, :])

            # PE warm-up: chained matmuls on zeros into the psum ring
            wps = ppool.tile([128, 4, 512], F32, tag="ps")
            for _ in range(NWARM):
                nc.tensor.matmul(
                    wps[:, 0, 0:128], scr[:, :], scr[:, :], start=True, stop=True
                )

            for ji, job in enumerate(JOBS):
                k = len(job)
                ps = ppool.tile([128, 4, 512], F32, tag="ps")
                for s in range(NSLOT):
                    st, sp = s == 0, s == NSLOT - 1
                    for hb, half in ((0, slice(0, 64)), (k, slice(64, 128))):
                        for ci, (p, ch) in enumerate(job):
                            nc.tensor.matmul(
                                ps[:, hb + ci, 0:448],
                                _lhs(half, s),
                                _pair_rhs(xst[p], half, ch * RCH, s),
                                start=st,
                                stop=sp,
                                perf_mode=DR,
                            )
                ot = opool.tile([128, 2, k, RCH, W_DIM], BF16, tag="ot")
                nc.scalar.activation(
                    out=ot[:],
                    in_=ps[:, 0 : 2 * k, 0:448],
                    func=SQRT,
                    bias=_w2v(),
                    scale=1.0 / WSCALE,
                )
                # one output DMA per contiguous per-pair chunk run; the
                # last (shared) job's two half-size DMAs drain in parallel
                # on both hardware queues (scalar is idle after the final
                # act), halving the closing transfer
                ci = 0
                nrun = 0
                while ci < k:
                    p, ch = job[ci]
                    cj = ci + 1
                    while cj < k and job[cj] == (p, ch + (cj - ci)):
                        cj += 1
                    nr = cj - ci
                    eng = (
                        nc.scalar
                        if (ji == len(JOBS) - 1 and nrun == 1)
                        else nc.sync
                    )
                    eng.dma_start(
                        out=out[p, :, :, ch * RCH : (ch + nr) * RCH, :],
                        in_=ot[:, :, ci:cj],
                    )
                    ci = cj
                    nrun += 1
    nc.compile()
    return nc


def _host_weights(W):
    """fp8 lhsT [128, 5, 2, 128]: k-tile pairs of -32*W taps (dup on both
    halves); slot (4,1) is the one-hot b' row (16 at k-rows 0-2);
    w2 = ||W[o]||^2 + BCENTER f32 (the b' recentering folds into the bias),
    byte-packed after the fp8 weights so both ride ONE DMA."""
    W = np.asarray(W, np.float32)
    lhs = np.zeros((128, NSLOT, 2, 128), np.float32)
    cidx = np.arange(C)

    def tapw(kh, kw):
        return (-2.0 * WSCALE * W[:, cidx * 9 + kh * 3 + kw]).T  # [C, O]

    taps = [(kh, kw) for kh in range(3) for kw in range(3)]
    for s in range(NSLOT):
        lhs[0:64, s, 0, :] = tapw(*taps[2 * s])
        lhs[64:128, s, 0, :] = tapw(*taps[2 * s])
        if s < NSLOT - 1:
            lhs[0:64, s, 1, :] = tapw(*taps[2 * s + 1])
            lhs[64:128, s, 1, :] = tapw(*taps[2 * s + 1])
    lhs[0:3, NSLOT - 1, 1, :] = WSCALE  # b' 3-term fp8 expansion rows
    lhs[64:67, NSLOT - 1, 1, :] = WSCALE
    w2 = ((W * W).sum(axis=1) + BCENTER).astype(np.float32).reshape(128, 1)
    packed = np.empty((128, NSLOT * 256 + 4), NP_FP8)
    packed[:, : NSLOT * 256] = lhs.astype(NP_FP8).reshape(128, NSLOT * 256)
    packed[:, NSLOT * 256 :] = w2.view(np.uint8).view(NP_FP8)
    return packed


def get_program():
    global _PROGRAM
    if _PROGRAM is None:
        _PROGRAM = _build_program()
    return _PROGRAM


def make_in_maps(x, W):
    x = np.asarray(x, np.float32)
    xpad = np.zeros((N, C, HP, WP), np.float32)
    xpad[:, :, 1 : H + 1, 1 : W_DIM + 1] = x
    xq8 = xpad.astype(NP_FP8).reshape(NCORES, NPAIR, 2, C, HP, WP)

    # b' = 3x3 box of the channel-sum of x^2, recentered: ||p||^2 - BCENTER.
    ss = (xpad * xpad).sum(axis=1)  # [N, HP, WP]
    b = np.zeros((N, H, W_DIM), np.float32)
    for di in range(3):
        for dj in range(3):
            b += ss[:, di : di + H, dj : dj + W_DIM]
    bplane = np.zeros((N, HP, WP), np.float32)
    bplane[:, 2:HP, 2:WP] = b - BCENTER
    # 3-term greedy fp8 expansion: b' = b1+b2+b3, residual error < 0.25
    parts = []
    r = bplane
    for _ in range(3):
        t = np.clip(r, -240.0, 240.0).astype(NP_FP8)
        parts.append(t)
        r = r - t.astype(np.float32)
    # [NCORES, NPAIR, img, term, HP*WP]
    bq8 = np.stack(parts, axis=1).reshape(NCORES, NPAIR, 2, 3, HP * WP)

    # x tile images: img A channels on partitions 0-63, img B on 64-127
    xq = np.zeros((NCORES, NPAIR, 128, HP * WP), NP_FP8)
    xq[:, :, 0:C] = xq8[:, :, 0].reshape(NCORES, NPAIR, C, HP * WP)
    xq[:, :, C : 2 * C] = xq8[:, :, 1].reshape(NCORES, NPAIR, C, HP * WP)

    lw = _host_weights(W)
    return [
        {"xq": xq[i], "bq": bq8[i], "lw": lw}
        for i in range(NCORES)
    ]


def kernel(x, W):
    nc = get_program()
    in_maps = make_in_maps(x, W)
    res = run_bass_kernel_spmd(nc, in_maps, list(range(NCORES)))
    outs = []
    for i in range(NCORES):
        o = np.asarray(res.results[i]["out"])  # [NPAIR, O, 2, H, W] bf16
        outs.append(o.transpose(0, 2, 1, 3, 4).reshape(NL, O, H, W_DIM))
    return np.concatenate(outs, axis=0).astype(np.float32)


# revision 49
# speedup vs baseline: 1.0175x; 1.0175x over previous
"""Trainium2 Bass kernel for Conv2D_DT (distance-transform conv).

d(n,o,h,w) = || patch(n,:,h,w) - W[o,:] ||_2  with 3x3/pad1 im2col patches.

Strategy (8 NeuronCores, data-parallel over batch, 4 images/core):
  - the compute-heavy cross term -2 p.w runs as fp8 DoubleRow matmuls at
    the PE's full fp8 rate: each matmul contracts TWO 3x3 taps at once
    (k-tile pair), using hand-built access patterns whose k-tile dim
    strides between the two shifted x windows.  9 taps -> 4 tap-pair
    matmuls + 1 final matmul that pairs tap8 with the ||p||^2 term: its
    second k-tile reads a precomputed b' = ||p||^2 - 576 row (partitions
    0-2/64-66, one-hot weight rows of 16), so the whole quadratic form
    accumulates in PSUM in 5 DoubleRow matmuls per chunk-image.
  - b' = 3x3-box(channel-sum(x^2)) - 576 is computed on host (f32,
    exact) and shipped as a tiny 3-term fp8 expansion (6 partitions per
    pair); the remaining 122 partitions of the b' plane only need to be
    FINITE (their lhsT rows are zero), so they are memset on the
    otherwise-idle vector/gpsimd engines instead of DMAing zeros.
  - image pairs: image A channels on SBUF partitions 0-63, B on 64-127.
  - input DMAs are split between the two hardware DGE queues (sync and
    scalar; gpsimd's software path only carries tiny b' rows) so their
    ~0.65us-per-issue cost is paid in parallel right after the NEFF
    preamble; outputs issue on sync (idle during the stream).
    The pair-0 b' rows lead their queues (half A on scalar, half B on
    gpsimd): job 0's slot-4 stop matmuls wait on them, and any
    late-modeled writer makes the Tile scheduler defer those stop
    matmuls deep into job 1, which inflates the first epilogue act's
    wait threshold (observed 61 -> 53 across these placements) and
    opens a PE gap at the job1->job2 psum-ring handoff.
  - the f32 w2 bias is byte-packed onto the tail of the weights tensor
    (one DMA, one completion semaphore) because a [128,1] f32 transfer
    is 128 straggling 4-byte packets that would gate the first act.
  - warm-up matmuls on a zeroed scratch tile keep the PE busy from right
    after the preamble until the first x rows land, so the HAM clock
    gate reaches 8/8 before (or shortly after) the real tap stream.
  - a dummy Sqrt activation at the head pulls the lazy ~1.3us ACT table
    load off the first epilogue's critical path.
  - the work is cut into 7 UNIFORM psum jobs of 20 matmuls / one
    full-size act (psum tile [128,4,512] = 4 banks, ring of 2): the two
    odd leftover chunks of the two image pairs merge into one shared
    job, so no short job ever bubbles the psum ring.  The shared job
    runs LAST: its output is naturally two half-size DMAs, which drain
    in parallel on the sync + scalar hardware queues at the tail.
  - epilogue: ONE ScalarE op per job covering both images:
    out = Sqrt(psum/16 + (w2+576)) -> bf16, then one output DMA per
    contiguous chunk run.
"""

import sys

_REPO = "/opt/trn_rl_repo"
if _REPO not in sys.path:
    sys.path.insert(0, _REPO)

import ml_dtypes
import numpy as np

import concourse.bass as bass  # noqa: F401
import concourse.mybir as mybir
import concourse.tile as tile
from concourse import bacc
from concourse.bass_utils import run_bass_kernel_spmd

# Problem geometry (hardcoded per harness contract).
N, C, H, W_DIM, O = 32, 64, 56, 56, 128
NCORES = 8
NL = N // NCORES  # images per core
NPAIR = NL // 2  # image pairs per core
HP = WP = 58  # zero-padded spatial dims
RCH = 8  # output rows per PSUM chunk slot
WSCALE = 16.0  # fp8 W pre-scale; undone by epilogue scale=1/WSCALE
BCENTER = 576.0  # E[||p||^2]; recentering keeps b' in fp8 range
NWARM = 26  # PE warm-up matmuls (HAM clock-gate ramp)
NSLOT = 5  # DoubleRow k-tile pairs: (t0,t1)(t2,t3)(t4,t5)(t6,t7)(t8,b)

F32 = mybir.dt.float32
BF16 = mybir.dt.bfloat16
U32 = mybir.dt.uint32
FP8 = mybir.dt.float8e4
NP_FP8 = ml_dtypes.float8_e4m3

# PSUM jobs: each is a list of (pair, chunk) entries sharing one 4-bank
# psum tile + one epilogue act.  The two odd leftover chunks (pair-0
# chunk 6 and pair-1 chunk 0) are merged into ONE shared k=2 job, so all
# 7 jobs are uniform 20-matmul / full-size-act units - no short-group
# pipeline bubbles in the psum ring.
JOBS = (
    ((0, 0), (0, 1)),
    ((0, 2), (0, 3)),
    ((0, 4), (0, 5)),
    ((1, 1), (1, 2)),
    ((1, 3), (1, 4)),
    ((1, 5), (1, 6)),
    ((0, 6), (1, 0)),
)
# tap-A (kh,kw) of each DoubleRow pair, and the k-tile-dim element stride
# from tap A's window to tap B's (tap index t=(kh,kw) offset = kh*WP+kw;
# slot 4 jumps from x (slot 0) to the b' plane (slot 1) of the same tile)
TAPA = ((0, 0), (0, 2), (1, 1), (2, 0), (2, 2))
DELTA = (1, WP - 2, 1, 1, HP * WP)

_PROGRAM = None


def _pair_rhs(t, half, lh, s):
    """rhs AP [64, 2, RCH, 56] for DoubleRow pair s: dim1 walks from tap A's
    shifted window to tap B's (stride DELTA[s]) inside tile t (free dims
    flat: slot*HP*WP + row*WP + col)."""
    kh, kw = TAPA[s]
    off = (lh + kh) * WP + kw
    ap = t[half, 0, off : off + 56]
    l = ap.ap
    l.insert(1, (DELTA[s], 2))
    l.insert(2, (WP, RCH))
    return ap


def _build_program():
    nc = bacc.Bacc(
        "TRN2",
        target_bir_lowering=False,
        debug=False,
        enable_asserts=False,
        num_devices=NCORES,
    )
    xq = nc.dram_tensor("xq", [NPAIR, 128, HP * WP], FP8, kind="ExternalInput")
    bq = nc.dram_tensor("bq", [NPAIR, 2, 3, HP * WP], FP8, kind="ExternalInput")
    # weights + per-partition f32 w2 bias packed into one tensor so the
    # bias lands with the (early, sync-queue) weight DMA instead of as
    # 128 straggling 4-byte packets that gate the first epilogue act
    lw = nc.dram_tensor("lw", [128, NSLOT * 256 + 4], FP8, kind="ExternalInput")
    out = nc.dram_tensor("out", [NPAIR, O, 2, H, W_DIM], BF16, kind="ExternalOutput")

    DR = mybir.MatmulPerfMode.DoubleRow
    SQRT = mybir.ActivationFunctionType.Sqrt

    with tile.TileContext(nc) as tc:
        with (
            tc.tile_pool(name="const", bufs=1) as cpool,
            tc.tile_pool(name="imgs", bufs=2) as ipool,
            tc.tile_pool(name="outs", bufs=6) as opool,
            tc.tile_pool(name="psum", bufs=2, space="PSUM") as ppool,
        ):
            # scratch for PE warm-up (zeroed so no NaNs reach the PE)
            scr = cpool.tile([128, 128], FP8)
            nc.vector.memset(scr[:], 0)
            dum = cpool.tile([128, 1], F32)
            dumo = cpool.tile([128, 1], F32)

            xst = []
            for p in range(NPAIR):
                xsp = ipool.tile([128, 2, HP * WP], FP8, tag="xs")
                xst.append(xsp)

            lwt = cpool.tile([128, NSLOT * 256 + 4], FP8)

            def _lhs(half, sl):
                ap = lwt[half, 256 * sl : 256 * (sl + 1)]
                l = ap.ap
                l.pop(1)
                l.insert(1, (128, 2))
                l.insert(2, (1, 128))
                return ap

            def _w2v():
                return lwt[:, NSLOT * 256 : NSLOT * 256 + 4].bitcast(F32)

            # Input marshaling. Only sync and scalar have hardware DGE
            # queues (gpsimd DMAs descriptor-gen in software - slow), so
            # the big transfers ride those two, issue cost in parallel.
            # b'-plane zero fills emitted BEFORE the b' row DMAs: program
            # order is dependency order, the rows must land after the fill.
            nc.vector.memset(xst[0][0:64, 1, :].bitcast(U32), 0)
            nc.vector.memset(xst[0][64:128, 1, :].bitcast(U32), 0)
            nc.vector.memset(dum[:], 0)
            nc.vector.memset(xst[1][0:64, 1, :].bitcast(U32), 0)
            nc.vector.memset(xst[1][64:128, 1, :].bitcast(U32), 0)
            # sync: weights first (gate the first LDWEIGHTS), then pair-1 x
            nc.sync.dma_start(out=lwt[:], in_=lw[:, :])
            nc.sync.dma_start(out=xst[1][:, 0, :], in_=xq[1, :, :])
            # scalar: pair-0 x (first-group rows first), w2 bias; then a
            # dummy Sqrt so the ~1.3us ACT table load runs during the DMA
            # head, not before the first epilogue act
            # pair-0 b' rows lead their queues: job 0's slot-4 stop
            # matmuls wait on them, and if they model late the Tile
            # scheduler defers those stop MMs deep into job 1, stalling
            # the act chain.  Half A leads scalar; half B leads gpsimd
            # (software DGE, but it starts early and models early).
            nc.scalar.dma_start(out=xst[0][0:3, 1, :], in_=bq[0, 0, :, :])
            nc.gpsimd.dma_start(out=xst[0][64:67, 1, :], in_=bq[0, 1, :, :])
            nc.scalar.dma_start(
                out=xst[0][:, 0, 0 : 20 * WP], in_=xq[0, :, 0 : 20 * WP]
            )
            nc.scalar.dma_start(
                out=xst[0][:, 0, 20 * WP : HP * WP], in_=xq[0, :, 20 * WP : HP * WP]
            )
            nc.scalar.activation(
                out=dumo[:], in_=dum[:], func=SQRT, bias=dum[:], scale=1.0 / WSCALE
            )
            # gpsimd: pair-1 b' rows (tiny; software path is fine here)
            nc.gpsimd.dma_start(out=xst[1][0:3, 1, :], in_=bq[1, 0, :, :])
            nc.gpsimd.dma_start(out=xst[1][64:67, 1, :], in_=bq[1, 1, :, :])

            # PE warm-up: chained matmuls on zeros into the psum ring
            wps = ppool.tile([128, 4, 512], F32, tag="ps")
            for _ in range(NWARM):
                nc.tensor.matmul(
                    wps[:, 0, 0:128], scr[:, :], scr[:, :], start=True, stop=True
                )

            for ji, job in enumerate(JOBS):
                k = len(job)
                ps = ppool.tile([128, 4, 512], F32, tag="ps")
                for s in range(NSLOT):
                    st, sp = s == 0, s == NSLOT - 1
                    for hb, half in ((0, slice(0, 64)), (k, slice(64, 128))):
                        for ci, (p, ch) in enumerate(job):
                            nc.tensor.matmul(
                                ps[:, hb + ci, 0:448],
                                _lhs(half, s),
                                _pair_rhs(xst[p], half, ch * RCH, s),
                                start=st,
                                stop=sp,
                                perf_mode=DR,
                            )
                ot = opool.tile([128, 2, k, RCH, W_DIM], BF16, tag="ot")
                nc.scalar.activation(
                    out=ot[:],
                    in_=ps[:, 0 : 2 * k, 0:448],
                    func=SQRT,
                    bias=_w2v(),
                    scale=1.0 / WSCALE,
                )
                # one output DMA per contiguous per-pair chunk run; the
                # last (shared) job's two half-size DMAs drain in parallel
                # on both hardware queues (scalar is idle after the final
                # act), halving the closing transfer
                ci = 0
                nrun = 0
                while ci < k:
                    p, ch = job[ci]
                    cj = ci + 1
                    while cj < k and job[cj] == (p, ch + (cj - ci)):
                        cj += 1
                    nr = cj - ci
                    eng = (
                        nc.scalar
                        if (ji == len(JOBS) - 1 and nrun == 1)
                        else nc.sync
                    )
                    eng.dma_start(
                        out=out[p, :, :, ch * RCH : (ch + nr) * RCH, :],
                        in_=ot[:, :, ci:cj],
                    )
                    ci = cj
                    nrun += 1
    nc.compile()
    return nc


def _host_weights(W):
    """fp8 lhsT [128, 5, 2, 128]: k-tile pairs of -32*W taps (dup on both
    halves); slot (4,1) is the one-hot b' row (16 at k-rows 0-2);
    w2 = ||W[o]||^2 + BCENTER f32 (the b' recentering folds into the bias),
    byte-packed after the fp8 weights so both ride ONE DMA."""
    W = np.asarray(W, np.float32)
    lhs = np.zeros((128, NSLOT, 2, 128), np.float32)
    cidx = np.arange(C)

    def tapw(kh, kw):
        return (-2.0 * WSCALE * W[:, cidx * 9 + kh * 3 + kw]).T  # [C, O]

    taps = [(kh, kw) for kh in range(3) for kw in range(3)]
    for s in range(NSLOT):
        lhs[0:64, s, 0, :] = tapw(*taps[2 * s])
        lhs[64:128, s, 0, :] = tapw(*taps[2 * s])
        if s < NSLOT - 1:
            lhs[0:64, s, 1, :] = tapw(*taps[2 * s + 1])
            lhs[64:128, s, 1, :] = tapw(*taps[2 * s + 1])
    lhs[0:3, NSLOT - 1, 1, :] = WSCALE  # b' 3-term fp8 expansion rows
    lhs[64:67, NSLOT - 1, 1, :] = WSCALE
    w2 = ((W * W).sum(axis=1) + BCENTER).astype(np.float32).reshape(128, 1)
    packed = np.empty((128, NSLOT * 256 + 4), NP_FP8)
    packed[:, : NSLOT * 256] = lhs.astype(NP_FP8).reshape(128, NSLOT * 256)
    packed[:, NSLOT * 256 :] = w2.view(np.uint8).view(NP_FP8)
    return packed


def get_program():
    global _PROGRAM
    if _PROGRAM is None:
        _PROGRAM = _build_program()
    return _PROGRAM


def make_in_maps(x, W):
    x = np.asarray(x, np.float32)
    xpad = np.zeros((N, C, HP, WP), np.float32)
    xpad[:, :, 1 : H + 1, 1 : W_DIM + 1] = x
    xq8 = xpad.astype(NP_FP8).reshape(NCORES, NPAIR, 2, C, HP, WP)

    # b' = 3x3 box of the channel-sum of x^2, recentered: ||p||^2 - BCENTER.
    ss = (xpad * xpad).sum(axis=1)  # [N, HP, WP]
    b = np.zeros((N, H, W_DIM), np.float32)
    for di in range(3):
        for dj in range(3):
            b += ss[:, di : di + H, dj : dj + W_DIM]
    bplane = np.zeros((N, HP, WP), np.float32)
    bplane[:, 2:HP, 2:WP] = b - BCENTER
    # 3-term greedy fp8 expansion: b' = b1+b2+b3, residual error < 0.25
    parts = []
    r = bplane
    for _ in range(3):
        t = np.clip(r, -240.0, 240.0).astype(NP_FP8)
        parts.append(t)
        r = r - t.astype(np.float32)
    # [NCORES, NPAIR, img, term, HP*WP]
    bq8 = np.stack(parts, axis=1).reshape(NCORES, NPAIR, 2, 3, HP * WP)

    # x tile images: img A channels on partitions 0-63, img B on 64-127
    xq = np.zeros((NCORES, NPAIR, 128, HP * WP), NP_FP8)
    xq[:, :, 0:C] = xq8[:, :, 0].reshape(NCORES, NPAIR, C, HP * WP)
    xq[:, :, C : 2 * C] = xq8[:, :, 1].reshape(NCORES, NPAIR, C, HP * WP)

    lw = _host_weights(W)
    return [
        {"xq": xq[i], "bq": bq8[i], "lw": lw}
        for i in range(NCORES)
    ]


def kernel(x, W):
    nc = get_program()
    in_maps = make_in_maps(x, W)
    res = run_bass_kernel_spmd(nc, in_maps, list(range(NCORES)))
    outs = []
    for i in range(NCORES):
        o = np.asarray(res.results[i]["out"])  # [NPAIR, O, 2, H, W] bf16
        outs.append(o.transpose(0, 2, 1, 3, 4).reshape(NL, O, H, W_DIM))
    return np.concatenate(outs, axis=0).astype(np.float32)
